# revision 22
# baseline (speedup 1.0000x reference)
"""Trainium2 Bass kernel for nn_ChemicalDevelopment (drag-scan + separable
Gaussian blur + mask-combine + 3x3 channel coupling + tanh saturation).

Self-contained: hardcodes shapes/sharding. Shards the W (column) axis across
8 NeuronCores (512 cols each, 1-col blur halo); each core processes its
full-height column slab independently (no collectives).

v4: channels deinterleaved into 3 planes of 514 cols; host ships
xq = x*SINV as fp16 (the 1/SINV unscale is baked into the scan/blur
weights, so s,h come out unscaled). Row tiles of 128 partitions overlap by
2 rows (stride 126) so the vertical blur (radius 1; dropped taps ~5e-4
mass) needs no neighbour-tile halo matmuls.

Per (tile b, plane p): PE fp16 matmuls (1 cyc/row): scan s = T'x + U'x_prev
-> ps_sh[:,0:512]; 2D blur h = sum_d Bd x[:,shifted] -> ps_sh[:,512:1024]
(3 shifted matmuls PSUM-accumulated); one ACT scatter-copy -> fp16 s3|h3.
Per tile: xs3 = DMA row-shifted copy of xq (= SINV*x at out rows);
DVE: d3 = h3-s3, pp3 = xs3*d3 (fp16 2x, full-width); GPSIMD: u3 = s3+pp3;
PE: per out channel j, identity matmul seeds ps_v with xs3_j then 3
diagonal matmuls accumulate -SINV*C[i,j]*u3_i; one ACT tanh(ps_v) -> out.
The *3 supply limit is folded into the host-side gather.
"""
import numpy as np

H_FULL = 4096
W_FULL = 4096
NCORES = 8
WS = W_FULL // NCORES      # 512 columns per core
RH = 1                     # truncated blur radius (taps |e|>1 ~5e-4 mass)
WP = WS + 2 * RH           # padded plane width (514)
P = 128                    # partition block (rows)
OUT_R = P - 2 * RH         # 126 output rows per tile
NB = -(-H_FULL // OUT_R)   # 33 tiles
PAD_T = RH                 # one zero row above the image
H_PAD = OUT_R * (NB - 1) + P  # 4160 padded rows
PAD_B = H_PAD - H_FULL - PAD_T
HIST = 62                  # scan history rows from previous tile
SIGMA_SOFT = 2.0
SIGMA_HARD = 0.5
D_MAX = 3.0
SINV = 1.0 / (D_MAX + 1e-6)
DMX = D_MAX + 1e-6         # baked into T/U/B to unscale xq
F = 3 * WP                 # SBUF x-tile free width (3*514=1542)
FC = 3 * WS                # output width (1536)

_NC_CACHE = {}


def _taps():
    # identical arithmetic to the reference (f32), truncated to radius RH
    # and renormalized
    x = np.arange(-12, 13, dtype=np.float32)
    k = np.exp(np.float32(-0.5) * (x / np.float32(SIGMA_HARD)) ** 2)
    k = k / k.sum()
    kept = k[12 - RH:12 + RH + 1].astype(np.float64)
    return kept / kept.sum()


def _matrices():
    d = np.exp(-1.0 / SIGMA_SOFT)
    scale = (1.0 - d) * DMX
    k = np.arange(P)[:, None]          # in-tile partition
    r = np.arange(OUT_R)[None, :]      # out row (tile partition r+1)
    e = r + 1 - k
    with np.errstate(under="ignore"):
        T = np.where(e >= 0, scale * d ** np.clip(e, 0, None), 0.0)
        h = np.arange(HIST)[:, None]   # xp partition 64+h
        U = scale * d ** (r + 63 - h)
    kt = _taps()
    band = np.where(np.abs(k - r - 1) <= RH,
                    kt[np.clip(k - r - 1 + RH, 0, 2 * RH)], 0.0)
    Bc = kt[RH] * band * DMX        # center tap
    Bs = kt[0] * band * DMX         # symmetric side taps (applied to x_l+x_r)
    f = lambda a: np.ascontiguousarray(a, np.float16)
    return f(T), f(U), f(Bc), f(Bs)


def _build_nc(nb, last_rows):
    import concourse.bacc as bacc
    import concourse.mybir as mybir
    from concourse.tile import TileContext

    f32 = mybir.dt.float32
    f16 = mybir.dt.float16
    AO = mybir.AluOpType

    T, U, Bc, Bs = _matrices()
    NW = 4                           # weight blocks: T, U, Bc, Bs
    wconst_np = np.zeros((P, NW * OUT_R), np.float16)
    wconst_np[:, 0:OUT_R] = T
    wconst_np[64:64 + HIST, OUT_R:2 * OUT_R] = U
    wconst_np[:, 2 * OUT_R:3 * OUT_R] = Bc
    wconst_np[:, 3 * OUT_R:4 * OUT_R] = Bs
    ident_np = np.ascontiguousarray(np.eye(OUT_R, dtype=np.float16))

    nc = bacc.Bacc(trn_type="TRN2", debug=False)
    hx = nc.dram_tensor("x", [H_PAD, F], f16, kind="ExternalInput")
    hcm = nc.dram_tensor("cmat", [P, 9], f32, kind="ExternalInput")
    hy = nc.dram_tensor("y", [H_FULL, FC], f16, kind="ExternalOutput")
    hconst = nc.inline_tensor(wconst_np, name="wconst")
    hident = nc.inline_tensor(ident_np, name="ident")

    with TileContext(nc) as tc:
        with tc.tile_pool(name="wpool", bufs=1) as wpool, \
             tc.tile_pool(name="xpool", bufs=4) as xpool, \
             tc.tile_pool(name="spool", bufs=2) as spool, \
             tc.tile_pool(name="upool", bufs=2) as upool, \
             tc.tile_pool(name="opool", bufs=2) as opool, \
             tc.tile_pool(name="pshpool", bufs=2, space="PSUM") as pshpool, \
             tc.tile_pool(name="psvpool", bufs=1, space="PSUM") as psvpool:

            wconst = wpool.tile([P, NW * OUT_R], f16, name="wconst_t")
            nc.sync.dma_start(out=wconst, in_=hconst[:, :])
            wT = wconst[:, 0:OUT_R]
            wU = wconst[64:64 + HIST, OUT_R:2 * OUT_R]
            wBc = wconst[:, 2 * OUT_R:3 * OUT_R]
            wBs = wconst[:, 3 * OUT_R:4 * OUT_R]
            ident = wpool.tile([OUT_R, OUT_R], f16, name="ident_t")
            nc.sync.dma_start(out=ident, in_=hident[:, :])

            # negc[p, 3i+j] = -C[i,j]*SINV on every partition (host bcast)
            cmsb = wpool.tile([P, 9], f32, name="cmsb")
            nc.sync.dma_start(out=cmsb, in_=hcm[:, :])
            negc = wpool.tile([P, 9], f32, name="negc")
            nc.scalar.mul(negc, cmsb, -SINV)
            # 9 diagonal mix-weight tiles diag(-C[i,j]*SINV)
            wmix = []
            for kk in range(9):
                dg = wpool.tile([OUT_R, OUT_R], f16, name=f"wmix{kk}")
                nc.vector.tensor_scalar_mul(out=dg, in0=ident,
                                            scalar1=negc[0:OUT_R, kk:kk + 1])
                wmix.append(dg)

            x_tiles = [None] * nb

            def load(b):
                xt = xpool.tile([P, F], f16, name=f"x{b}", tag="x")
                nc.sync.dma_start(out=xt, in_=hx[b * OUT_R:b * OUT_R + P, :])
                x_tiles[b] = xt

            def process(b):
                xb = x_tiles[b]
                xp = x_tiles[b - 1] if b > 0 else None

                sh3 = spool.tile([OUT_R, 2 * FC], f16, name=f"sh3_{b}",
                                 tag="sh3")
                sh3v = sh3.rearrange("p (g x) -> p g x", g=2)
                # xs3 = row-shifted xq at out rows (one strided DMA;
                # DMA moves across partitions freely)
                xs3 = spool.tile([OUT_R, FC], f16, name=f"xs3_{b}", tag="xs3")
                xbp = xb.rearrange("p (g w) -> p g w", g=3)
                nc.sync.dma_start(
                    out=xs3.rearrange("p (g w) -> p g w", g=3),
                    in_=xbp[RH:RH + OUT_R, :, RH:RH + WS])

                # symmetric blur side taps: xpair = x_left + x_right
                xpair = spool.tile([P, FC], f16, name=f"xpair_{b}",
                                   tag="xpair")
                nc.vector.tensor_add(
                    out=xpair.rearrange("p (g w) -> p g w", g=3),
                    in0=xbp[:, :, 0:WS], in1=xbp[:, :, 2 * RH:2 * RH + WS])

                for p in range(3):
                    c0 = p * WP
                    ctr = slice(c0 + RH, c0 + RH + WS)
                    ps_sh = pshpool.tile([P, 2 * WS], f32,
                                         name=f"ps_sh{b}_{p}", tag="ps_sh")
                    nc.tensor.matmul(out=ps_sh[0:OUT_R, 0:WS], lhsT=wT,
                                     rhs=xb[:, ctr],
                                     start=True, stop=(xp is None))
                    if xp is not None:
                        nc.tensor.matmul(out=ps_sh[0:OUT_R, 0:WS], lhsT=wU,
                                         rhs=xp[64:64 + HIST, ctr],
                                         start=False, stop=True,
                                         tile_position=(64, 0))
                    nc.tensor.matmul(out=ps_sh[0:OUT_R, WS:2 * WS],
                                     lhsT=wBc, rhs=xb[:, ctr],
                                     start=True, stop=False)
                    nc.tensor.matmul(out=ps_sh[0:OUT_R, WS:2 * WS],
                                     lhsT=wBs,
                                     rhs=xpair[:, p * WS:(p + 1) * WS],
                                     start=False, stop=True)
                    # scatter s|h into plane-grouped sh3
                    nc.scalar.copy(
                        out=sh3v[:, :, p * WS:(p + 1) * WS],
                        in_=ps_sh[0:OUT_R, :].rearrange("p (g x) -> p g x",
                                                        g=2))

                s3 = sh3[:, 0:FC]
                h3 = sh3[:, FC:2 * FC]
                d3 = spool.tile([OUT_R, FC], f16, name=f"d3_{b}", tag="d3")
                nc.vector.tensor_sub(out=d3, in0=h3, in1=s3)
                pp3 = spool.tile([OUT_R, FC], f16, name=f"pp3_{b}", tag="pp3")
                nc.vector.tensor_mul(out=pp3, in0=xs3, in1=d3)
                u3 = upool.tile([OUT_R, FC], f16, name=f"u3_{b}", tag="u3")
                nc.gpsimd.tensor_tensor(out=u3, in0=s3, in1=pp3, op=AO.add)

                # channel mix on PE: seed with xs3_j, accumulate -cs_ij*u_i
                ps_v = psvpool.tile([P, FC], f32, name=f"ps_v{b}", tag="ps_v")
                for j in range(3):
                    nc.tensor.matmul(
                        out=ps_v[0:OUT_R, j * WS:(j + 1) * WS],
                        lhsT=ident, rhs=xs3[:, j * WS:(j + 1) * WS],
                        start=True, stop=False)
                    for i in range(3):
                        nc.tensor.matmul(
                            out=ps_v[0:OUT_R, j * WS:(j + 1) * WS],
                            lhsT=wmix[3 * i + j],
                            rhs=u3[:, i * WS:(i + 1) * WS],
                            start=False, stop=(i == 2))

                ot = opool.tile([OUT_R, FC], f16, name=f"o{b}", tag="o")
                nc.scalar.activation(out=ot, in_=ps_v[0:OUT_R, :],
                                     func=mybir.ActivationFunctionType.Tanh)

                rows = last_rows if b == nb - 1 else OUT_R
                nc.sync.dma_start(out=hy[b * OUT_R:b * OUT_R + rows, :],
                                  in_=ot[0:rows, :])

            load(0)
            if nb > 1:
                load(1)
            for b in range(nb):
                if b + 2 < nb:
                    load(b + 2)
                process(b)

    nc.finalize()
    return nc


def _get_nc():
    key = (NB, H_FULL - OUT_R * (NB - 1))
    if key not in _NC_CACHE:
        _NC_CACHE[key] = _build_nc(NB, H_FULL - OUT_R * (NB - 1))
    return _NC_CACHE[key]


def make_in_maps(D_macro, coupling_matrix):
    D = np.asarray(D_macro, dtype=np.float32)
    C = np.asarray(coupling_matrix, np.float32).reshape(1, 9)
    Cb = np.ascontiguousarray(np.broadcast_to(C, (P, 9)))
    Dp = np.pad(D * np.float32(SINV),
                ((PAD_T, PAD_B), (RH, RH), (0, 0))).astype(np.float16)
    DT = np.ascontiguousarray(Dp.transpose(0, 2, 1))  # (H_PAD, 3, W+2RH)
    in_maps = []
    for m in range(NCORES):
        sl = np.ascontiguousarray(
            DT[:, :, m * WS:m * WS + WP]).reshape(H_PAD, F)
        in_maps.append({"x": sl, "cmat": Cb})
    return in_maps


def kernel(D_macro, coupling_matrix):
    from concourse.bass_utils import run_bass_kernel_spmd

    in_maps = make_in_maps(D_macro, coupling_matrix)
    nc = _get_nc()
    res = run_bass_kernel_spmd(nc, in_maps, core_ids=list(range(NCORES)))
    # supply_limit (*3) and fp16->fp32 upcast folded into the gather
    outs = [(r["y"].reshape(H_FULL, 3, WS).astype(np.float32) * 3.0)
            .transpose(0, 2, 1) for r in res.results]
    return np.ascontiguousarray(np.concatenate(outs, axis=1))


# revision 26
# speedup vs baseline: 1.1411x; 1.1411x over previous
"""Trainium2 Bass kernel for nn_ChemicalDevelopment (drag-scan + separable
Gaussian blur + mask-combine + 3x3 channel coupling + tanh saturation).

Self-contained: hardcodes shapes/sharding. Shards the W (column) axis across
8 NeuronCores (512 cols each, 1-col blur halo); each core processes its
full-height column slab independently (no collectives).

v4: channels deinterleaved into 3 planes of 514 cols; host ships
xq = x*SINV as fp16 (the 1/SINV unscale is baked into the scan/blur
weights, so s,h come out unscaled). Row tiles of 128 partitions overlap by
2 rows (stride 126) so the vertical blur (radius 1; dropped taps ~5e-4
mass) needs no neighbour-tile halo matmuls.

Per (tile b, plane p): PE fp16 matmuls (1 cyc/row): scan s = T'x + U'x_prev
-> ps_sh[:,0:512]; 2D blur h = sum_d Bd x[:,shifted] -> ps_sh[:,512:1024]
(3 shifted matmuls PSUM-accumulated); one ACT scatter-copy -> fp16 s3|h3.
Per tile: xs3 = DMA row-shifted copy of xq (= SINV*x at out rows);
DVE: d3 = h3-s3, pp3 = xs3*d3 (fp16 2x, full-width); GPSIMD: u3 = s3+pp3;
PE: per out channel j, identity matmul seeds ps_v with xs3_j then 3
diagonal matmuls accumulate -SINV*C[i,j]*u3_i; one ACT tanh(ps_v) -> out.
The *3 supply limit is folded into the host-side gather.
"""
import numpy as np

H_FULL = 4096
W_FULL = 4096
NCORES = 8
WS = W_FULL // NCORES      # 512 columns per core
RH = 1                     # truncated blur radius (taps |e|>1 ~5e-4 mass)
WP = WS + 2 * RH           # padded plane width (514)
P = 128                    # partition block (rows)
OUT_R = P - 2 * RH         # 126 output rows per tile
NB = -(-H_FULL // OUT_R)   # 33 tiles
PAD_T = RH                 # one zero row above the image
H_PAD = OUT_R * (NB - 1) + P  # 4160 padded rows
PAD_B = H_PAD - H_FULL - PAD_T
HIST = 62                  # scan history rows from previous tile
SIGMA_SOFT = 2.0
SIGMA_HARD = 0.5
D_MAX = 3.0
SINV = 1.0 / (D_MAX + 1e-6)
DMX = D_MAX + 1e-6         # baked into T/U/B to unscale xq
F = 3 * WP                 # SBUF x-tile free width (3*514=1542)
FC = 3 * WS                # output width (1536)

_NC_CACHE = {}


def _taps():
    # identical arithmetic to the reference (f32), truncated to radius RH
    # and renormalized
    x = np.arange(-12, 13, dtype=np.float32)
    k = np.exp(np.float32(-0.5) * (x / np.float32(SIGMA_HARD)) ** 2)
    k = k / k.sum()
    kept = k[12 - RH:12 + RH + 1].astype(np.float64)
    return kept / kept.sum()


def _matrices():
    d = np.exp(-1.0 / SIGMA_SOFT)
    scale = (1.0 - d) * DMX
    k = np.arange(P)[:, None]          # in-tile partition
    r = np.arange(OUT_R)[None, :]      # out row (tile partition r+1)
    e = r + 1 - k
    with np.errstate(under="ignore"):
        T = np.where(e >= 0, scale * d ** np.clip(e, 0, None), 0.0)
        h = np.arange(HIST)[:, None]   # xp partition 64+h
        U = scale * d ** (r + 63 - h)
    kt = _taps()
    B = []
    for dd in range(-RH, RH + 1):
        band = np.where(np.abs(k - r - 1) <= RH,
                        kt[np.clip(k - r - 1 + RH, 0, 2 * RH)], 0.0)
        B.append(kt[dd + RH] * band * DMX)
    f = lambda a: np.ascontiguousarray(a, np.float16)
    return f(T), f(U), [f(b) for b in B]


def _build_nc(nb, last_rows):
    import concourse.bacc as bacc
    import concourse.mybir as mybir
    from concourse.tile import TileContext

    f32 = mybir.dt.float32
    f16 = mybir.dt.float16
    AO = mybir.AluOpType

    T, U, B = _matrices()
    NW = 2 + len(B)                  # weight blocks: T, U, B*3
    wconst_np = np.zeros((P, NW * OUT_R), np.float16)
    wconst_np[:, 0:OUT_R] = T
    wconst_np[64:64 + HIST, OUT_R:2 * OUT_R] = U
    for i, b in enumerate(B):
        wconst_np[:, (2 + i) * OUT_R:(3 + i) * OUT_R] = b
    ident_np = np.ascontiguousarray(np.eye(OUT_R, dtype=np.float16))

    nc = bacc.Bacc(trn_type="TRN2", debug=False)
    hx = nc.dram_tensor("x", [H_PAD, F], f16, kind="ExternalInput")
    hcm = nc.dram_tensor("cmat", [P, 9], f32, kind="ExternalInput")
    hy = nc.dram_tensor("y", [H_FULL, FC], f16, kind="ExternalOutput")
    hconst = nc.inline_tensor(wconst_np, name="wconst")
    hident = nc.inline_tensor(ident_np, name="ident")

    with TileContext(nc) as tc:
        with tc.tile_pool(name="wpool", bufs=1) as wpool, \
             tc.tile_pool(name="xpool", bufs=4) as xpool, \
             tc.tile_pool(name="spool", bufs=2) as spool, \
             tc.tile_pool(name="upool", bufs=2) as upool, \
             tc.tile_pool(name="opool", bufs=2) as opool, \
             tc.tile_pool(name="pshpool", bufs=2, space="PSUM") as pshpool, \
             tc.tile_pool(name="psvpool", bufs=1, space="PSUM") as psvpool:

            wconst = wpool.tile([P, NW * OUT_R], f16, name="wconst_t")
            nc.sync.dma_start(out=wconst, in_=hconst[:, :])
            wT = wconst[:, 0:OUT_R]
            wU = wconst[64:64 + HIST, OUT_R:2 * OUT_R]
            wB = [wconst[:, (2 + i) * OUT_R:(3 + i) * OUT_R]
                  for i in range(len(B))]
            ident = wpool.tile([OUT_R, OUT_R], f16, name="ident_t")
            nc.sync.dma_start(out=ident, in_=hident[:, :])

            # negc[p, 3i+j] = -C[i,j]*SINV on every partition (host bcast)
            cmsb = wpool.tile([P, 9], f32, name="cmsb")
            nc.sync.dma_start(out=cmsb, in_=hcm[:, :])
            negc = wpool.tile([P, 9], f32, name="negc")
            nc.scalar.mul(negc, cmsb, -SINV)
            # 9 diagonal mix-weight tiles diag(-C[i,j]*SINV)
            wmix = []
            for kk in range(9):
                dg = wpool.tile([OUT_R, OUT_R], f16, name=f"wmix{kk}")
                nc.vector.tensor_scalar_mul(out=dg, in0=ident,
                                            scalar1=negc[0:OUT_R, kk:kk + 1])
                wmix.append(dg)

            x_tiles = [None] * nb

            def load(b):
                xt = xpool.tile([P, F], f16, name=f"x{b}", tag="x")
                nc.sync.dma_start(out=xt, in_=hx[b * OUT_R:b * OUT_R + P, :])
                x_tiles[b] = xt

            def process(b):
                xb = x_tiles[b]
                xp = x_tiles[b - 1] if b > 0 else None

                sh3 = spool.tile([OUT_R, 2 * FC], f16, name=f"sh3_{b}",
                                 tag="sh3")
                sh3v = sh3.rearrange("p (g x) -> p g x", g=2)
                # xs3 = row-shifted xq at out rows (one strided DMA;
                # DMA moves across partitions freely)
                xs3 = spool.tile([OUT_R, FC], f16, name=f"xs3_{b}", tag="xs3")
                xbp = xb.rearrange("p (g w) -> p g w", g=3)
                nc.sync.dma_start(
                    out=xs3.rearrange("p (g w) -> p g w", g=3),
                    in_=xbp[RH:RH + OUT_R, :, RH:RH + WS])

                for p in range(3):
                    c0 = p * WP
                    ctr = slice(c0 + RH, c0 + RH + WS)
                    ps_sh = pshpool.tile([P, 2 * WS], f32,
                                         name=f"ps_sh{b}_{p}", tag="ps_sh")
                    nc.tensor.matmul(out=ps_sh[0:OUT_R, 0:WS], lhsT=wT,
                                     rhs=xb[:, ctr],
                                     start=True, stop=(xp is None))
                    if xp is not None:
                        nc.tensor.matmul(out=ps_sh[0:OUT_R, 0:WS], lhsT=wU,
                                         rhs=xp[64:64 + HIST, ctr],
                                         start=False, stop=True,
                                         tile_position=(64, 0))
                    for i in range(len(B)):
                        dd = i - RH
                        sl = slice(c0 + RH + dd, c0 + RH + dd + WS)
                        nc.tensor.matmul(out=ps_sh[0:OUT_R, WS:2 * WS],
                                         lhsT=wB[i], rhs=xb[:, sl],
                                         start=(i == 0),
                                         stop=(i == len(B) - 1))
                    # scatter s|h into plane-grouped sh3
                    nc.scalar.copy(
                        out=sh3v[:, :, p * WS:(p + 1) * WS],
                        in_=ps_sh[0:OUT_R, :].rearrange("p (g x) -> p g x",
                                                        g=2))

                s3 = sh3[:, 0:FC]
                h3 = sh3[:, FC:2 * FC]
                d3 = spool.tile([OUT_R, FC], f16, name=f"d3_{b}", tag="d3")
                nc.vector.tensor_sub(out=d3, in0=h3, in1=s3)
                pp3 = spool.tile([OUT_R, FC], f16, name=f"pp3_{b}", tag="pp3")
                nc.vector.tensor_mul(out=pp3, in0=xs3, in1=d3)
                u3 = upool.tile([OUT_R, FC], f16, name=f"u3_{b}", tag="u3")
                nc.gpsimd.tensor_tensor(out=u3, in0=s3, in1=pp3, op=AO.add)

                # channel mix on PE: seed with xs3_j, accumulate -cs_ij*u_i
                ps_v = psvpool.tile([P, FC], f32, name=f"ps_v{b}", tag="ps_v")
                for j in range(3):
                    nc.tensor.matmul(
                        out=ps_v[0:OUT_R, j * WS:(j + 1) * WS],
                        lhsT=ident, rhs=xs3[:, j * WS:(j + 1) * WS],
                        start=True, stop=False)
                    for i in range(3):
                        nc.tensor.matmul(
                            out=ps_v[0:OUT_R, j * WS:(j + 1) * WS],
                            lhsT=wmix[3 * i + j],
                            rhs=u3[:, i * WS:(i + 1) * WS],
                            start=False, stop=(i == 2))

                ot = opool.tile([OUT_R, FC], f16, name=f"o{b}", tag="o")
                nc.scalar.activation(out=ot, in_=ps_v[0:OUT_R, :],
                                     func=mybir.ActivationFunctionType.Tanh)

                rows = last_rows if b == nb - 1 else OUT_R
                nc.sync.dma_start(out=hy[b * OUT_R:b * OUT_R + rows, :],
                                  in_=ot[0:rows, :])

            load(0)
            if nb > 1:
                load(1)
            for b in range(nb):
                if b + 2 < nb:
                    load(b + 2)
                process(b)

    nc.finalize()
    return nc


def _get_nc():
    key = (NB, H_FULL - OUT_R * (NB - 1))
    if key not in _NC_CACHE:
        _NC_CACHE[key] = _build_nc(NB, H_FULL - OUT_R * (NB - 1))
    return _NC_CACHE[key]


def make_in_maps(D_macro, coupling_matrix):
    D = np.asarray(D_macro, dtype=np.float32)
    C = np.asarray(coupling_matrix, np.float32).reshape(1, 9)
    Cb = np.ascontiguousarray(np.broadcast_to(C, (P, 9)))
    Dp = np.pad(D * np.float32(SINV),
                ((PAD_T, PAD_B), (RH, RH), (0, 0))).astype(np.float16)
    DT = np.ascontiguousarray(Dp.transpose(0, 2, 1))  # (H_PAD, 3, W+2RH)
    in_maps = []
    for m in range(NCORES):
        sl = np.ascontiguousarray(
            DT[:, :, m * WS:m * WS + WP]).reshape(H_PAD, F)
        in_maps.append({"x": sl, "cmat": Cb})
    return in_maps


def kernel(D_macro, coupling_matrix):
    from concourse.bass_utils import run_bass_kernel_spmd

    in_maps = make_in_maps(D_macro, coupling_matrix)
    nc = _get_nc()
    res = run_bass_kernel_spmd(nc, in_maps, core_ids=list(range(NCORES)))
    # supply_limit (*3) and fp16->fp32 upcast folded into the gather
    outs = [(r["y"].reshape(H_FULL, 3, WS).astype(np.float32) * 3.0)
            .transpose(0, 2, 1) for r in res.results]
    return np.ascontiguousarray(np.concatenate(outs, axis=1))
